# revision 74
# baseline (speedup 1.0000x reference)
"""Trainium2 Bass kernel v3: dense transformer block, fp8 attention + fp8 MLP.

Sequence-parallel over 8 cores (core = (batch, quarter)), zero collectives,
identical SPMD program. Changes vs v2 (620us -> target ~400us):
- MLP in fp8: fc1 via DoubleRowSwInterleave (weights stationary, ln2T fp8
  moving pairs), fc2 via DoubleRow (hT fp8 pair stationaries, fp8 weight
  moving). Halves MLP matmul count and stream time.
- Softmax denominator folded into V as a 65th ones-column (64.0): the AV
  DoubleRow matmul produces [65, TQ] = av rows 0:64 + den row 64. Kills all
  ones-stationary den matmuls (128 N=512 matmuls/core).
- V proj and attn proj use 128-wide stationaries (full-array DR) instead of
  64: halves their matmul count.
- LN1 computed in two halves with K proj interleaved per half; QK weight
  stationaries preloaded during LN1 so projections never DMA-wait.
- exp computed 50/50 on Scalar (native Exp, bias=-30 kills invalid keys)
  and Vector (custom DVE op: exp(z) ~ (1+z/64)^64, 6 chained squares).
- Bias algebra: K-bias softmax-invariant (dropped); V/proj/fc2 biases shift
  the residual uniformly per token -> LayerNorm-invariant, added on host.
  Only Q and fc1 biases stay on-device (free via activation bias).
- fp8 weights upscaled x64 on host (fp8e4 denormal avoidance); the exp
  scale 1/(64*64*8) folds the two x64 and the 1/sqrt(dh).
"""

import numpy as np
from contextlib import ExitStack

import ml_dtypes

_BF16 = ml_dtypes.bfloat16
_E4 = ml_dtypes.float8_e4m3

FULL_CFG = dict(B=2, T=2048, D=1024, H=16, F=4096, EPS=1e-5)
NCORES = 8
DEBUG_DUMP = False
JCH = 4          # sequence chunks per batch
TQ = 512         # own tokens per core
SW = 64.0        # fp8 weight upscale
ESC = 1.0 / (SW * SW * 8.0)   # exp scale: undo 64*64, apply 1/sqrt(dh)
MASKB = -30.0    # additive bias killing invalid keys pre-exp
DHV = 65         # head dim in vp tiles: 64 value dims + 1 ones column


# ------------------------------------------------------------- custom DVE exp
def _get_exp_op():
    import concourse.dve_ops as dve_ops
    from concourse.dve_spec import Spec, Src0, C0, C1, sq, lower
    from concourse.dve_uop import DveOpSpec

    name = "EXP_SQ64_ANT"
    for op in dve_ops.OPS:
        if op.name == name:
            return op

    def _ref(in0, in1, c0, c1, c2):
        u = in0 * c0 + c1
        for _ in range(6):
            u = u * u
        return u

    body = Src0 * C0 + C1
    for _ in range(6):
        body = sq(body)
    op = dve_ops.DveOp(name, Spec(body=body, reference=_ref), subdim=False,
                       uops_sha={})
    dve_ops.OPS.append(op)
    dve_ops.CUSTOM_DVE_SPECS[name] = op.spec
    dve_ops._SUB_OPCODE_FOR_NAME[name] = (
        dve_ops._CUSTOM_DVE_ROW_BASE + len(dve_ops.OPS) - 1)
    assert dve_ops._SUB_OPCODE_FOR_NAME[name] < 0x20
    for ver in ("v3", "v4"):
        spec = DveOpSpec(name=name, opcode=dve_ops.get_dve_sub_opcode(name),
                         uops=lower(op.spec, ver=ver),
                         rd1_en=False)
        op.uops_sha[ver] = spec.sha(ver)
    return op


def _dims(cfg):
    B, T, D, H, F = cfg["B"], cfg["T"], cfg["D"], cfg["H"], cfg["F"]
    DH = D // H
    KT = T // 128            # keytiles
    DJ = TQ // 128           # own toktiles
    NX = D // 128            # xdim chunks
    NFC = F // 128           # fc-col tiles
    return B, T, D, H, F, DH, KT, DJ, NX, NFC


# ---------------------------------------------------------------- builder
def build_program(cfg):
    import concourse.tile as tile
    from concourse import bacc, mybir

    B, T, D, H, F, DH, KT, DJ, NX, NFC = _dims(cfg)
    TH = T // 2              # ln1T half size (tokens)
    f32 = mybir.dt.float32
    bf16 = mybir.dt.bfloat16
    fp8 = mybir.dt.float8e4
    AF = mybir.ActivationFunctionType
    OP = mybir.AluOpType
    DR = mybir.MatmulPerfMode.DoubleRow
    DRS = mybir.MatmulPerfMode.DoubleRowSwInterleave
    EXP_OP = _get_exp_op()

    nc = bacc.Bacc("TRN2", target_bir_lowering=False, debug=False,
                   num_devices=NCORES)

    def din(name, shape, dt=fp8):
        return nc.dram_tensor(name, list(shape), dt, kind="ExternalInput").ap()

    xb = din("xb", (KT, 128, D), f32)
    xres = din("xres", (DJ, 128, D), f32)
    wq8 = din("wq8", (128, 8, 4, 256))
    wk8 = din("wk8", (128, 8, 4, 256))
    wv8 = din("wv8", (128, 4, 2, 2, 512))
    wp8 = din("wp8", (128, 4, 2, 2, 512))
    bq64i = din("bq64", (128, NX), f32)
    wfc8 = din("wfc8", (NFC, 128, 4, 256))
    w2p = din("w2p", (NFC // 2, 128, 2, 2, 512), bf16)
    bfci = din("bfc", (128, NFC), f32)
    kvsci = din("kvsc", (128, KT), f32)
    kvdvi = din("kvdv", (128, KT), f32)
    maskbi = din("maskb", (128, DJ, TQ), bf16)
    maskAi = din("maskA", (128, 3, 256), bf16)
    identi = din("ident", (128, 128), bf16)
    identci = din("identc", (128, 128), bf16)
    out_d = nc.dram_tensor("out", [DJ, 128, D], f32, kind="ExternalOutput").ap()
    if DEBUG_DUMP:
        dbg_x2 = nc.dram_tensor("dbg_x2", [DJ, 128, D], f32,
                                kind="ExternalOutput").ap()
        dbg_vp = nc.dram_tensor("dbg_vp", [KT // 2, 128, 2, H, DHV], fp8,
                                kind="ExternalOutput").ap()
        dbg_aT = nc.dram_tensor("dbg_aT", [DJ, 128, 2, TQ], fp8,
                                kind="ExternalOutput").ap()
        dbg_kT = nc.dram_tensor("dbg_kT", [128, T], fp8,
                                kind="ExternalOutput").ap()

    with tile.TileContext(nc) as tc, ExitStack() as ctx:
        def pool(name, bufs, space="SBUF"):
            return ctx.enter_context(tc.tile_pool(name=name, bufs=bufs, space=space))

        consts = pool("consts", 1)
        xpool = pool("xpool", 3)
        stats = pool("stats", 8)
        lnbf = pool("lnbf", 2)
        ln1T_p = pool("ln1T", 2)
        kT_p = pool("kT", NX)
        qT_p = pool("qT", NX)
        vp_p = pool("vp", KT // 2)
        p_p = pool("ppool", 6)
        aT_p = pool("aT", DJ)
        rsp = pool("rsp", 4)
        avp = pool("avp", 3)
        x2_p = pool("x2", DJ)
        ln2T_p = pool("ln2T", NX // 2)
        hT_p = pool("hT", NFC // 2)
        outp = pool("outp", 2)
        wqk_pool = pool("wqkp", 2)           # preloaded QK fp8 stationaries
        wv_pool = pool("wvp", 2)             # wv8 / wp8 moving tiles
        wk_pool = pool("wkp", 4)             # fc1 weights (streamed)
        wr_pool = pool("wrp", 3)             # fc2 moving tiles (streamed)
        psS = pool("psS", 3, space="PSUM")   # scores [128,2,512] f32 (2 banks)
        psA = pool("psA", 2, space="PSUM")   # [65..128,512] f32 accs / av

        # ---- consts (tiles allocated now; bulky DMAs deferred into the LN1
        # loop so the x-tile stream is never stuck behind them)
        ident = consts.tile([128, 128], bf16, tag="ident", name="ident")
        nc.sync.dma_start(ident[:], identi[:, :])
        # HAM warmup: ~4us of back-to-back dummy transposes at t~1us flips
        # the PE clock gate to 8/8 before the first real transposes (~8us);
        # without this the first ~34us of the kernel runs at 1.2 GHz.
        warm = psS.tile([128, 4, 128], bf16, tag="s", name="warm")
        for w in range(36):
            nc.tensor.transpose(warm[:, w % 4, :], ident[:, :], ident[:, :])
        epst = consts.tile([128, 1], f32, tag="epst", name="epst")
        nc.gpsimd.memset(epst[:], cfg["EPS"])
        kvsc = consts.tile([128, KT], f32, tag="kvsc", name="kvsc")
        kvdv = consts.tile([128, KT], f32, tag="kvdv", name="kvdv")
        maskb = consts.tile([128, DJ, TQ], bf16, tag="maskb", name="maskb")
        maskA = consts.tile([128, 3, 256], bf16, tag="maskA", name="maskA")
        identc = consts.tile([128, 128], bf16, tag="identc", name="identc")
        bq64 = consts.tile([128, NX], f32, tag="bq64", name="bq64")
        bfc = consts.tile([128, NFC], f32, tag="bfc", name="bfc")

        # persistent fp8 stationaries/moving tiles, one batched DMA each
        wk_all = wqk_pool.tile([128, NX, 4, 256], fp8, tag="wqk", name="wk")
        wq_all = wqk_pool.tile([128, NX, 4, 256], fp8, tag="wqk", name="wq")
        wv_all = wv_pool.tile([128, 4, 2, 2, 512], fp8, tag="wv", name="wv")

        def deferred_dmas(tt):
            if tt == 0:
                nc.sync.dma_start(wv_all[:], wv8[:, :, :, :, :])
                nc.sync.dma_start(wk_all[:], wk8[:, :, :, :])
            elif tt == 3:
                nc.sync.dma_start(wq_all[:], wq8[:, :, :, :])
            elif tt == 5:
                nc.sync.dma_start(kvsc[:], kvsci[:, :])
                nc.sync.dma_start(kvdv[:], kvdvi[:, :])
                nc.sync.dma_start(maskb[:], maskbi[:, :, :])
                nc.sync.dma_start(maskA[:], maskAi[:, :, :])
                nc.sync.dma_start(identc[:], identci[:, :])
                nc.sync.dma_start(bq64[:], bq64i[:, :])
                nc.sync.dma_start(bfc[:], bfci[:, :])

        def wk_t(kd, t):
            return wk_all[:, kd, t, :]

        def wq_t(kd, t):
            return wq_all[:, kd, t, :]

        def wv_t(t, c):
            return wv_all[:, t, c, :, :]

        # ---------------- helpers
        def ln_statsA(x_t):
            """stage A: bn stats + sqrt(var) (vector, scalar)."""
            st = stats.tile([128, 2, 6], f32, tag="bnst")
            xr = x_t.rearrange("p (s c) -> p s c", s=2)
            for s in range(2):
                nc.vector.bn_stats(st[:, s, :], xr[:, s, :])
            mv = stats.tile([128, 2], f32, tag="bnmv", name="bnmv")
            nc.vector.bn_aggr(mv[:, :], st[:, :, :])
            sd = stats.tile([128, 1], f32, tag="rstd", name="rstd")
            nc.scalar.activation(sd[:, :], mv[:, 1:2], AF.Sqrt, bias=epst[:, :])
            return mv, sd

        def ln_statsB(mv, sd):
            """stage B: recip + -mu*rstd (vector)."""
            nc.vector.reciprocal(sd[:, :], sd[:, :])
            nmr = stats.tile([128, 1], f32, tag="nmr", name="nmr")
            nc.vector.scalar_tensor_tensor(nmr[:, :], mv[:, 0:1], -1.0,
                                           sd[:, :], OP.mult, OP.mult)
            return nmr, sd

        def psum_copy(eng_i, dst, src):
            # gpsimd cannot access PSUM: alternate vector/scalar only
            if eng_i % 2 == 0:
                nc.vector.tensor_copy(dst, src)
            else:
                nc.scalar.copy(dst, src)

        # ---------------- phase 1: LN1 + transpose -> ln1T fp8 (two halves),
        # with K proj / V proj for each half interleaved right after it so the
        # PE never drains while the second half's LN chain runs.
        ln1T = [ln1T_p.tile([128, NX, TH], fp8, tag="ln1T", name="ln1T")
                for _ in range(2)]

        def ln1m(pair, c0, w):
            """moving AP [128, 2, w] for xdim pair at token offset c0."""
            half, off = divmod(c0, TH)
            assert off + w <= TH
            return ln1T[half][:, 2 * pair:2 * pair + 2, off:off + w]

        def ln1_tail(pt, px, pmv, psd):
            nmr, rstd = ln_statsB(pmv, psd)
            lt = lnbf.tile([128, D], bf16, tag="lnbf", name="lnbf")
            nc.scalar.activation(lt[:, :], px[:, :], AF.Identity,
                                 bias=nmr[:, :], scale=rstd[:, :])
            half, off = divmod(pt * 128, TH)
            # 4 transposes share one PSUM bank, drained by a single copy
            for g in range(2):
                tp = psS.tile([128, 4, 128], bf16, tag="s", name="tp")
                for i in range(4):
                    xc = 4 * g + i
                    nc.tensor.transpose(tp[:, i, :],
                                        lt[:, xc * 128:(xc + 1) * 128],
                                        ident[:, :])
                psum_copy(g + pt, ln1T[half][:, 4 * g:4 * g + 4, off:off + 128],
                          tp[:, :, :])

        kT = [kT_p.tile([128, T], fp8, tag="kT", name="kT") for _ in range(NX)]
        # vp[t2] holds keytile pair (2*t2, 2*t2+1): [128 tok, 2, H, DHV];
        # col 64 of each head is the ones column (written once from vones)
        vp = [vp_p.tile([128, 2, H, DHV], fp8, tag="vp", name="vp")
              for _ in range(KT // 2)]
        for t2 in range(KT // 2):
            # ones column for the in-V softmax denominator; memset, NOT a
            # DMA: a stride-65 byte-granular DMA shatters into 32K packets
            nc.gpsimd.memset(vp[t2][:, :, :, 64:65], SW)

        def k_proj_chunk(c):
            """K proj for token chunk c (needs ln1 tiles 4c..4c+3 done)."""
            for kd in range(NX):
                acc = psA.tile([128, 512], f32, tag="acc", name="kacc")
                for t in range(4):
                    nc.tensor.matmul(acc[:, :], wk_t(kd, t),
                                     ln1m(t, c * 512, 512),
                                     start=(t == 0), stop=(t == 3),
                                     perf_mode=DRS)
                psum_copy(kd, kT[kd][:, c * 512:(c + 1) * 512], acc[:, :])

        def v_proj_tile(tt):
            """V proj for keytile tt (fp8 DR, 128-wide stationary)."""
            vacc = psS.tile([128, 2, TQ], f32, tag="s", name="vacc")
            for u in range(4):
                st = ln1m(u, tt * 128, 128)
                for c in range(2):
                    nc.tensor.matmul(vacc[:, c, :], st, wv_t(u, c),
                                     start=(u == 0), stop=(u == 3),
                                     perf_mode=DR)
            dstv = vp[tt // 2]
            for c in range(2):
                a3 = vacc[:, c, :].rearrange("p (h c) -> p h c", c=DH)
                psum_copy(tt + c, dstv[:, tt % 2, c * 8:(c + 1) * 8, 0:DH], a3)

        ln_prev = None
        for tt in range(KT):
            x_t = xpool.tile([128, D], f32, tag="xt", name="xt")
            nc.sync.dma_start(x_t[:], xb[tt, :, :])
            cur = (tt, x_t, *ln_statsA(x_t))
            if ln_prev is not None:
                ln1_tail(*ln_prev)
                v_proj_tile(tt - 1)
            ln_prev = cur
            deferred_dmas(tt)
            if tt % 4 == 0 and tt > 0:   # chunk tt//4 - 1 fully written
                k_proj_chunk(tt // 4 - 1)
        ln1_tail(*ln_prev)
        v_proj_tile(KT - 1)
        k_proj_chunk(3)

        # ---------------- phase 2b: Q proj (own tokens, last 512)
        qT = [qT_p.tile([128, TQ], fp8, tag="qT", name="qT") for _ in range(NX)]
        q0 = T - TQ
        for kd in range(NX):
            acc = psA.tile([128, 512], f32, tag="acc", name="qacc")
            for t in range(4):
                nc.tensor.matmul(acc[:, :], wq_t(kd, t), ln1m(t, q0, TQ),
                                 start=(t == 0), stop=(t == 3), perf_mode=DRS)
            nc.scalar.activation(qT[kd][:, :], acc[:, :], AF.Identity,
                                 bias=bq64[:, kd:kd + 1])

        # prefetch attn-proj weights during attention
        wp_all = wv_pool.tile([128, 4, 2, 2, 512], fp8, tag="wv", name="wp")
        nc.sync.dma_start(wp_all[:], wp8[:, :, :, :, :])

        # ---------------- phase 3: attention
        # aT pair tiles: aTp[t][:, i, :] = attn-out dims of chunk 2t+i
        aTp = [aT_p.tile([128, 2, TQ], fp8, tag="aT", name="aT")
               for _ in range(DJ)]
        NPAIR = KT // 2
        # diagonal pairs (6,7) mid-stream: their mask DVE work stays clear of
        # the head-pair boundary where the deferred norm contends for DVE/psum
        # Mirrored-causality slot layout (host permutes keys): slot pairs
        # 0,1,2 = context below toktile 8 minus own piece A; pairs 3,4,5 =
        # context 8..15 minus own piece B; pair 6 = own piece A (queries
        # 0:256); pair 7 = own piece B (queries 256:512). Pairs 0,1,2,6 are
        # scored against ALL 512 queries (S1: piece-A validity via the maskA
        # matmul-add on columns 0:256); pairs 3,4,5,7 only against piece B's
        # 256 columns (S2) -- piece A never attends keys above toktile 8.
        PAIR_ORDER = [0, 1, 2, 6, 3, 4, 7, 5]

        def emit_av(ti, t, qo, qw, p0, p1, av0, av1, h0, h1):
            # av[0:64] = attn values; av[64] = denominator (ones col in V)
            st0 = vp[t][:, :, h0, :]
            st1 = vp[t][:, :, h1, :]
            st_, sp_ = ti == 0, ti == NPAIR - 1
            nc.tensor.matmul(av0[:, qo:qo + qw], st0, p0[:, :, qo:qo + qw],
                             start=st_, stop=sp_, perf_mode=DR,
                             skip_group_check=True)
            nc.tensor.matmul(av1[:, qo:qo + qw], st1, p1[:, :, qo:qo + qw],
                             start=st_, stop=sp_, perf_mode=DR,
                             skip_group_check=True)

        def head_norm_drain(h, av):
            """Free the av PSUM slot fast: den row to partition 0 (scalar),
            approx recip (vector; needs partition-0 SBUF src), av copy."""
            rs0 = rsp.tile([1, TQ], f32, tag="rs", name="rs0")
            nc.scalar.copy(rs0[:, :], av[64:65, :])
            rs = rsp.tile([1, TQ], f32, tag="rs", name="rs")
            nc.vector.reciprocal_approx_fast(rs[:, :], rs0[:, :])
            avs = avp.tile([64, TQ], bf16, tag="avs", name="avs")
            nc.scalar.copy(avs[:, :], av[0:64, :])
            return rs, avs

        def head_norm_apply(h, rs, avs):
            # ones column is 64.0: den = 64*sum(p), so recip gives
            # 1/(64 den) directly -- no separate 1/SW scaling pass
            rb = rsp.tile([64, TQ], f32, tag="rb", name="rb")
            nc.gpsimd.partition_broadcast(rb[:, :], rs[:, :], channels=64)
            t2, sl = (h // 2) // 2, (h // 2) % 2
            ro = (h % 2) * 64
            nc.vector.tensor_tensor(aTp[t2][ro:ro + 64, sl, :], avs[:, :],
                                    rb[:, :], op=OP.mult)

        prev = None
        pending_norm = []
        drained = []
        for hp in range(H // 2):
            h0, h1 = 2 * hp, 2 * hp + 1
            kd = hp
            av0 = psA.tile([DHV, TQ], f32, tag="acc", name="av0")[:, :]
            av1 = psA.tile([DHV, TQ], f32, tag="acc", name="av1")[:, :]
            for ti, t in enumerate(PAIR_ORDER):
                s2 = t in (3, 4, 5, 7)
                qo, qw = (256, 256) if s2 else (0, TQ)
                diagA, diagB = t == 6, t == 7
                amask = t in (0, 1, 2)
                sAB0 = psS.tile([128, 2, TQ], f32, tag="s", name="sAB0")
                sAB1 = psS.tile([128, 2, TQ], f32, tag="s", name="sAB1")
                # Diagonal pairs get the causal mask ADDED pre-exp on the
                # tensor engine (identc @ maskb = -15*65536 where invalid;
                # the exp scale turns that into -30). S1 context pairs get
                # the per-core piece-A validity mask on columns 0:256.
                has_add = diagA or diagB or amask
                for sAB, r0, r1 in ((sAB0, 0, DH), (sAB1, DH, 128)):
                    for i in range(2):
                        kt = 2 * t + i
                        nc.tensor.matmul(sAB[:, i, qo:qo + qw],
                                         kT[kd][r0:r1, kt * 128:(kt + 1) * 128],
                                         qT[kd][r0:r1, qo:qo + qw],
                                         start=True, stop=not has_add,
                                         skip_group_check=True)
                        if diagA:
                            nc.tensor.matmul(sAB[:, i, :], identc[:, :],
                                             maskb[:, i, :],
                                             start=False, stop=True,
                                             skip_group_check=True)
                        elif diagB:
                            nc.tensor.matmul(sAB[:, i, 256:512], identc[:, :],
                                             maskb[:, 2 + i, 256:512],
                                             start=False, stop=True,
                                             skip_group_check=True)
                        elif amask:
                            nc.tensor.matmul(sAB[:, i, 0:256], identc[:, :],
                                             maskA[:, t, :],
                                             start=False, stop=True,
                                             skip_group_check=True)
                # drain the head pair finishing at ti==0 BEFORE its PSUM
                # slots get reused by this hp's first AV matmuls (psA has
                # exactly 2 bufs: av0/av1)
                if ti == 1 and pending_norm:
                    drained = [(hn,) + head_norm_drain(hn, avn)
                               for hn, avn in pending_norm]
                    pending_norm = []
                if prev is not None:
                    emit_av(*prev)
                p0 = p_p.tile([128, 2, TQ], fp8, tag="p", name="p0")
                p1 = p_p.tile([128, 2, TQ], fp8, tag="p", name="p1")
                # keytile pairs are validity-homogeneous, so a pair shares
                # one bias column and one exp instruction per head. h0 on
                # scalar (native exp), h1 on vector (custom DVE op).
                kt = 2 * t
                nc.scalar.activation(p0[:, :, qo:qo + qw],
                                     sAB0[:, :, qo:qo + qw], AF.Exp,
                                     bias=kvsc[:, kt:kt + 1], scale=ESC)
                nc.vector._custom_dve(
                    EXP_OP, out=p1[:, :, qo:qo + qw],
                    in0=sAB1[:, :, qo:qo + qw],
                    s0=ESC / 64.0, s1=kvdv[:, kt:kt + 1])
                prev = (ti, t, qo, qw, p0, p1, av0, av1, h0, h1)
                if ti == 3 and drained:
                    for hn, rsn, avsn in drained:
                        head_norm_apply(hn, rsn, avsn)
                    drained = []
            pending_norm = [(h0, av0), (h1, av1)]
        emit_av(*prev)

        # ---------------- phase 4: attn proj (fp8 DR, 128-wide stationary).
        # Only aTp[3][:,1,:] (heads 14/15) is still pending at this point, so
        # tile 0's first six matmuls are emitted BEFORE the final drains:
        # they keep the PE busy through the softmax tail (no HAM re-throttle)
        # and overlap real work with the vector/scalar drain chain.
        x2 = [x2_p.tile([128, D], f32, tag="x2", name="x2") for _ in range(DJ)]
        pacc0 = psS.tile([128, 2, TQ], f32, tag="s", name="pacc")
        for t in range(3):
            st = aTp[t][:, :, 0:128]
            for c in range(2):
                nc.tensor.matmul(pacc0[:, c, :], st, wp_all[:, t, c, :, :],
                                 start=(t == 0), stop=False, perf_mode=DR)

        for hn, avn in pending_norm:
            hd = head_norm_drain(hn, avn)
            head_norm_apply(hn, *hd)

        for tt in range(DJ):
            if tt == 0:
                pacc = pacc0
                st = aTp[3][:, :, 0:128]
                for c in range(2):
                    nc.tensor.matmul(pacc[:, c, :], st, wp_all[:, 3, c, :, :],
                                     start=False, stop=True, perf_mode=DR)
            else:
                pacc = psS.tile([128, 2, TQ], f32, tag="s", name="pacc")
                for t in range(4):
                    st = aTp[t][:, :, tt * 128:(tt + 1) * 128]
                    for c in range(2):
                        nc.tensor.matmul(pacc[:, c, :], st,
                                         wp_all[:, t, c, :, :],
                                         start=(t == 0), stop=(t == 3),
                                         perf_mode=DR)
            for c in range(2):
                xr_t = outp.tile([128, 512], f32, tag="ot", name="xrt")
                nc.sync.dma_start(xr_t[:],
                                  xres[tt, :, c * 512:(c + 1) * 512])
                nc.vector.scalar_tensor_tensor(
                    x2[tt][:, c * 512:(c + 1) * 512],
                    pacc[:, c, :], 1.0 / SW, xr_t[:, :], OP.mult, OP.add)

        if DEBUG_DUMP:
            for tt in range(DJ):
                nc.sync.dma_start(dbg_x2[tt, :, :], x2[tt][:, :])
                nc.sync.dma_start(dbg_aT[tt, :, :, :], aTp[tt][:, :, :])
            for t2 in range(KT // 2):
                nc.sync.dma_start(dbg_vp[t2, :, :, :, :], vp[t2][:, :, :, :])
            nc.sync.dma_start(dbg_kT[:, :], kT[0][:, :])

        # ---------------- phase 5: LN2 + transpose -> ln2T fp8 pairs
        ln2T = [ln2T_p.tile([128, 2, TQ], fp8, tag="ln2T", name="ln2T")
                for _ in range(NX // 2)]
        ln2_pipe = [(tt, *ln_statsA(x2[tt])) for tt in range(DJ)]
        for (tt, pmv, psd) in ln2_pipe:
            nmr, rstd = ln_statsB(pmv, psd)
            lt = lnbf.tile([128, D], bf16, tag="lnbf", name="ln2bf")
            nc.scalar.activation(lt[:, :], x2[tt][:, :], AF.Identity,
                                 bias=nmr[:, :], scale=rstd[:, :])
            for g in range(2):
                tp = psS.tile([128, 4, 128], bf16, tag="s", name="tp2")
                for i in range(4):
                    xc = 4 * g + i
                    nc.tensor.transpose(tp[:, i, :],
                                        lt[:, xc * 128:(xc + 1) * 128],
                                        ident[:, :])
                for u in range(2):
                    psum_copy(g + u + tt,
                              ln2T[2 * g + u][:, :, tt * 128:(tt + 1) * 128],
                              tp[:, 2 * u:2 * u + 2, :])

        # ---------------- phase 6: fc1 + gelu -> hT bf16 pairs (fp8 DRS)
        # hT2[p][:, i, :] = gelu(fc1)[128*(2p+i) : 128*(2p+i+1), own tokens]
        hT2 = [hT_p.tile([128, 2, TQ], bf16, tag="hT", name="hT")
               for _ in range(NFC // 2)]
        for ft in range(NFC):
            acc = psS.tile([128, TQ], f32, tag="s", name="facc")
            wt = wk_pool.tile([128, 4, 256], fp8, tag="wk", name="wfc")
            nc.sync.dma_start(wt[:], wfc8[ft, :, :, :])
            for u in range(4):
                nc.tensor.matmul(acc[:, :], wt[:, u, :], ln2T[u][:, :, :],
                                 start=(u == 0), stop=(u == 3),
                                 perf_mode=DRS)
            nc.scalar.activation(hT2[ft // 2][:, ft % 2, :], acc[:, :],
                                 AF.Gelu, bias=bfc[:, ft:ft + 1],
                                 scale=1.0 / SW)

        # ---------------- phase 7: fc2 (bf16) + residual -> out. Two passes
        # by output-column half so the first half's drains + out DMAs overlap
        # the second half's matmuls.
        NP2 = NFC // 2
        for pc in range(2):
            a2 = [psS.tile([128, 2, TQ], f32, tag="s", name="f2accS")
                  for _ in range(2)]
            accs = [a2[ti // 2][:, ti % 2, :] for ti in range(DJ)]
            for p in range(NP2):
                mov = wr_pool.tile([128, 2, 512], bf16, tag="wr", name="wfc2")
                nc.sync.dma_start(mov[:], w2p[p, :, :, pc, :])
                for i in range(2):
                    for ti in range(DJ):
                        st = hT2[p][:, i, ti * 128:(ti + 1) * 128]
                        nc.tensor.matmul(
                            accs[ti], st, mov[:, i, :],
                            start=(p == 0 and i == 0),
                            stop=(p == NP2 - 1 and i == 1))
            for ti in range(DJ):
                o_t = outp.tile([128, 512], f32, tag="ot", name="ot")
                nc.vector.tensor_tensor(
                    o_t[:, :], accs[ti],
                    x2[ti][:, pc * 512:(pc + 1) * 512], op=OP.add)
                nc.sync.dma_start(out_d[ti, :, pc * 512:(pc + 1) * 512],
                                  o_t[:, :])

    nc.compile()
    return nc


# ---------------------------------------------------------------- host prep
def make_core_inputs(inputs, cfg):
    B, T, D, H, F, DH, KT, DJ, NX, NFC = _dims(cfg)
    x = np.asarray(inputs["x"], np.float32)
    ln1_w = np.asarray(inputs["ln1_w"], np.float32)
    ln1_b = np.asarray(inputs["ln1_b"], np.float32)
    attn_w = np.asarray(inputs["attn_w"], np.float32)
    attn_b = np.asarray(inputs["attn_b"], np.float32)
    proj_w = np.asarray(inputs["proj_w"], np.float32)
    proj_b = np.asarray(inputs["proj_b"], np.float32)
    ln2_w = np.asarray(inputs["ln2_w"], np.float32)
    ln2_b = np.asarray(inputs["ln2_b"], np.float32)
    fc_w = np.asarray(inputs["fc_w"], np.float32)
    fc_b = np.asarray(inputs["fc_b"], np.float32)
    fc2_w = np.asarray(inputs["fc2_w"], np.float32)
    fc2_b = np.asarray(inputs["fc2_b"], np.float32)

    Wqkv = ln1_w[:, None] * attn_w
    bqkv = attn_b + ln1_b @ attn_w
    Wq, Wk, Wv = Wqkv[:, :D], Wqkv[:, D:2 * D], Wqkv[:, 2 * D:]
    bq, bv = bqkv[:D], bqkv[2 * D:]
    Wfc = ln2_w[:, None] * fc_w
    bfc = fc_b + ln2_b @ fc_w
    # biases that shift the residual stream uniformly: LN2-invariant, added
    # on the host to every output row (exact for any runtime values)
    out_add = (bv @ proj_w + proj_b + fc2_b).astype(np.float32)

    def tile_st(w):
        # [D, M] -> [M/128, 4, 128, 256] fp8 SwInterleave stationaries, x SW:
        # per (out-chunk m, pair t): sb[:, 0::2] = W[2t-tile][:, ::-1],
        # sb[:, 1::2] = W[2t+1-tile][:, ::-1]
        M = w.shape[1]
        r = (w * SW).reshape(4, 2, 128, M // 128, 128)   # [t, i, k, m, c]
        out = np.zeros((M // 128, 4, 128, 256), np.float32)
        out[:, :, :, 0::2] = r[:, 0, :, :, ::-1].transpose(2, 0, 1, 3)
        out[:, :, :, 1::2] = r[:, 1, :, :, ::-1].transpose(2, 0, 1, 3)
        return np.ascontiguousarray(out).astype(_E4)

    def tile_mv(w):
        # [D, M] -> [4, M/512, 128, 2, 512] fp8 moving pairs, x SW
        M = w.shape[1]
        r = (w * SW).reshape(4, 2, 128, M // 512, 512)
        return np.ascontiguousarray(r.transpose(0, 3, 2, 1, 4)).astype(_E4)

    def part_major_st(w8):
        # [M/128, 4, 128, 256] -> [128, M/128, 4, 256] (partition-major for
        # one batched DMA into a [128, M/128, 4, 256] SBUF tile)
        return np.ascontiguousarray(w8.transpose(2, 0, 1, 3))

    def part_major_mv(w8):
        # [4, 2, 128, 2, 512] -> [128, 4, 2, 2, 512]
        return np.ascontiguousarray(w8.transpose(2, 0, 1, 3, 4))

    shared = dict(
        wq8=part_major_st(tile_st(Wq)), wk8=part_major_st(tile_st(Wk)),
        wv8=part_major_mv(tile_mv(Wv)), wp8=part_major_mv(tile_mv(proj_w)),
        bq64=np.ascontiguousarray((bq * SW).reshape(NX, 128).T, np.float32),
        wfc8=np.ascontiguousarray(tile_st(Wfc).transpose(0, 2, 1, 3)),
        w2p=np.ascontiguousarray(
            fc2_w.reshape(NFC // 2, 2, 128, 2, 512)
            .transpose(0, 2, 1, 3, 4)).astype(_BF16),
        bfc=np.ascontiguousarray(bfc.reshape(NFC, 128).T, np.float32),
        ident=np.eye(128, dtype=_BF16),
        identc=(65536.0 * np.eye(128)).astype(_BF16),
    )
    maskb = np.zeros((DJ, 128, TQ), np.float32)
    for g in range(DJ):
        for r in range(128):
            maskb[g, r, :g * 128 + r] = -15.0
    shared["maskb"] = np.ascontiguousarray(
        maskb.transpose(1, 0, 2)).astype(_BF16)

    in_maps = []
    for c in range(NCORES):
        b, j = c // (NCORES // B), c % (NCORES // B)
        # mirrored ownership: this core owns 256-token blocks j and 7-j
        # (toktiles oA and oB); keys are slot-permuted so the program's
        # fixed S1/S2 split covers exactly the causal context.
        xt = x[b].reshape(KT, 128, D)
        oA = [2 * j, 2 * j + 1]
        oB = [14 - 2 * j, 15 - 2 * j]
        lowO = [t for t in range(8) if t not in oA]
        highO = [t for t in range(8, 16) if t not in oB]
        slots = lowO + highO + oA + oB
        kvsc = np.zeros((KT,), np.float32)
        for p in range(3):     # S2 context pairs live at slots 6..11
            if highO[2 * p] >= 14 - 2 * j:
                kvsc[6 + 2 * p] = MASKB
                kvsc[7 + 2 * p] = MASKB
        maskA = np.zeros((3, 128, 256), np.float32)
        for p in range(3):     # S1 context pairs at slots 0..5, piece-A cols
            if lowO[2 * p] >= 2 * j:
                maskA[p] = -15.0
        m = dict(shared)
        m["xb"] = np.ascontiguousarray(xt[slots], np.float32)
        m["xres"] = np.ascontiguousarray(xt[oA + oB], np.float32)
        m["kvsc"] = np.ascontiguousarray(
            np.broadcast_to(kvsc[None, :], (128, KT)), np.float32)
        m["kvdv"] = np.ascontiguousarray(
            np.broadcast_to(1.0 + kvsc[None, :] / 64.0, (128, KT)), np.float32)
        m["maskA"] = np.ascontiguousarray(
            maskA.transpose(1, 0, 2)).astype(_BF16)
        in_maps.append(m)
    return in_maps, out_add


_CACHED = {}


def _get_program(cfg_key=None):
    if "nc" not in _CACHED:
        _CACHED["nc"] = build_program(FULL_CFG)
    return _CACHED["nc"]


def kernel(**inputs) -> np.ndarray:
    from concourse.bass_utils import run_bass_kernel_spmd

    cfg = FULL_CFG
    B, T, D = cfg["B"], cfg["T"], cfg["D"]
    nc = _get_program()
    in_maps, out_add = make_core_inputs(inputs, cfg)
    res = run_bass_kernel_spmd(nc, in_maps, core_ids=list(range(NCORES)))
    out = np.zeros((B, T, D), np.float32)
    for c in range(NCORES):
        b, j = c // (NCORES // B), c % (NCORES // B)
        r = res.results[c]["out"].reshape(TQ, D) + out_add[None, :]
        out[b, 2 * j * 128:(2 * j + 2) * 128] = r[0:256]
        out[b, (14 - 2 * j) * 128:(16 - 2 * j) * 128] = r[256:512]
    return out
